# revision 28
# baseline (speedup 1.0000x reference)
"""BagRE segment-mean + classifier kernel for 8 Trainium2 NeuronCores (fp8).

Problem:  hidden [262144, 256] f32, sorted bag_id [262144] i64 with 8192 bags,
          W [128, 256], b [128]  ->  logits [8192, 128] f32
          logits = (segment_mean(hidden, bag_id) @ W.T) + b

Strategy:
  bag_id is sorted -> rows per bag are contiguous.  Core k owns bags
  [1024k, 1024(k+1)), split into 8 blocks of 128 bags; block rows are padded
  to whole 128-row tiles (per-position max over cores so all 8 cores run one
  SPMD program).

  The hidden stream is quantized host-side to fp8 E3M4 with error feedback
  down each (bag, h) column, so the bag-sum error telescopes to one quantum
  (~4e-3 end-to-end rel err) while DMA bytes halve vs fp16.  Stream chunks
  alternate between the two HWDGE rings (sync / scalar) for double issue
  throughput.

  Per 128-row tile the PE accumulates one_hot(rel).T @ X into PSUM
  [128 bags, 256] f32.  One-hots (f16) come from DVE is_equal for most
  tiles, with a host-precomputed fp8 subset (single large upfront DMA) to
  keep the DVE off the critical path; the head of the stream is all-host so
  the PE starts before the DVE consts land.  ~16 warmup matmuls on a zero
  tile hold the PE HAM clock at 2.4 GHz through the DMA ramp.

  Finalize is a 3-stage pipeline, each stage one block behind the stream so
  the PE never waits on cross-engine latency: block j's PSUM sums are
  copied to SBUF (ACT) at block j's end; PE-transposed to [h, bags] (f16)
  at block j+1's end; classifier GEMM + fused recip/bias (DVE) + bf16
  store at block j+2's end.
"""

import os
import sys
import bisect
import contextlib
import numpy as np

try:
    import concourse.bass as bass  # noqa: F401
except Exception:  # pragma: no cover
    sys.path.insert(0, "/opt/trn_rl_repo")

import ml_dtypes
import concourse.bass as bass
import concourse.tile as tile
from concourse import mybir, bacc, masks
from concourse.bass_utils import run_bass_kernel_spmd

F8 = ml_dtypes.float8_e3m4

N = 262144
H = 256
C = 128
NUM_BAGS = 8192
NCORES = 8
BLOCK_BAGS = 128
BLOCKS_PER_CORE = NUM_BAGS // BLOCK_BAGS // NCORES   # 8

CH = int(os.environ.get("BK_CH", "12"))              # tiles per stream chunk
CH0 = int(os.environ.get("BK_CH0", "8"))             # first two (short) chunks
N_HOST_OH = int(os.environ.get("BK_NHOST", "112"))   # host-supplied one-hots
N_HEAD_OH = int(os.environ.get("BK_NHEAD", "24"))    # host one-hots up front
WARMUP_MM = int(os.environ.get("BK_WARMUP", "22"))

LAST_RESULTS = None
_prog_cache = {}


def _install_ntff_shim():
    """Register the axon NTFF profiling hook so trace=True works."""
    try:
        from antenv.axon_hooks import get_axon_ntff_profile_hook  # noqa: F401
        return True
    except Exception:
        pass
    try:
        import types
        import antenv
        from trn_agent_boot.trn_boot import _ntff_profile_via_ctypes

        hook = _ntff_profile_via_ctypes("/opt/axon/libaxon_pjrt.so")
        if hook is None:
            return False
        mod = types.ModuleType("antenv.axon_hooks")
        mod._hook = hook
        mod.get_axon_ntff_profile_hook = lambda: mod._hook
        mod.set_axon_ntff_profile_hook = lambda h: setattr(mod, "_hook", h)
        sys.modules["antenv.axon_hooks"] = mod
        antenv.axon_hooks = mod
        import concourse.bass_utils as bu

        orig_upload = bu.upload_artifacts

        def _safe_upload(tmpdir):
            try:
                return orig_upload(tmpdir)
            except Exception:
                return tmpdir

        bu.upload_artifacts = _safe_upload
        return True
    except Exception:
        return False


def _tile_assignment(T):
    """Per-tile one-hot source: 'h' host, 'v' DVE.

    The first N_HEAD_OH tiles are host tiles so the PE can start before the
    DVE's consts have landed; the rest spread evenly over the tail.
    """
    src = ['v'] * T
    nh = min(N_HOST_OH, T)
    head = min(N_HEAD_OH, nh)
    for t in range(head):
        src[t] = 'h'
    rest = nh - head
    Tr = T - head
    if rest > 0:
        for q in range(Tr):
            if (q * rest) // Tr != ((q + 1) * rest) // Tr:
                src[head + q] = 'h'
    return tuple(src)


def _build_program(pos_tblks):
    T = sum(pos_tblks)
    offs = [0]
    for tb in pos_tblks:
        offs.append(offs[-1] + tb)
    src = _tile_assignment(T)
    host_slot = np.cumsum([1 if s == 'h' else 0 for s in src]) - 1
    chunks = [(0, min(CH0, T))]
    if chunks[-1][1] < T:
        chunks.append((chunks[-1][1], min(chunks[-1][1] + CH0, T)))
    while chunks[-1][1] < T:
        chunks.append((chunks[-1][1], min(chunks[-1][1] + CH, T)))

    f32 = mybir.dt.float32
    f16 = mybir.dt.float16
    bf16 = mybir.dt.bfloat16
    f8e3 = mybir.dt.float8e3

    nc = bacc.Bacc(trn_type="TRN2", target_bir_lowering=False, debug=False)
    hid = nc.dram_tensor("hid", [128, T * H], f8e3, kind="ExternalInput").ap()
    n_h = sum(1 for s in src if s == 'h')
    oh_d = None
    if n_h:
        oh_d = nc.dram_tensor("oh", [128, n_h * 128], f8e3,
                              kind="ExternalInput").ap()
    # cst32: [relT (T) | nrelT (T) | b (C) | recip (8)]
    CW = 2 * T + C + BLOCKS_PER_CORE
    cst32 = nc.dram_tensor("cst32", [128, CW], f32, kind="ExternalInput").ap()
    wt = nc.dram_tensor("wt", [128, 2 * C], f16, kind="ExternalInput").ap()
    iota = nc.dram_tensor("iota", [128, 128], f16, kind="ExternalInput").ap()
    out = nc.dram_tensor("out", [BLOCKS_PER_CORE, 128, C], bf16,
                         kind="ExternalOutput").ap()

    with tile.TileContext(nc) as tc:
        with contextlib.ExitStack() as ctx:
            consts = ctx.enter_context(tc.tile_pool(name="consts", bufs=1))
            hid_pool = ctx.enter_context(tc.tile_pool(name="hid", bufs=10))
            a_pool = ctx.enter_context(tc.tile_pool(name="onehot", bufs=8))
            sums_pool = ctx.enter_context(tc.tile_pool(name="sums", bufs=3))
            sT_pool = ctx.enter_context(tc.tile_pool(name="sT", bufs=6))
            ob_pool = ctx.enter_context(tc.tile_pool(name="ob", bufs=3))
            psum_s = ctx.enter_context(
                tc.tile_pool(name="psum_s", bufs=3, space="PSUM"))
            psum_t = ctx.enter_context(
                tc.tile_pool(name="psum_t", bufs=2, space="PSUM"))
            psum_o = ctx.enter_context(
                tc.tile_pool(name="psum_o", bufs=2, space="PSUM"))
            psum_w = ctx.enter_context(
                tc.tile_pool(name="psum_w", bufs=1, space="PSUM"))

            # --- PE warmup: keep HAM at 2.4 GHz while DMA ramps ---------
            wz = consts.tile([128, 128], f16)
            nc.vector.memset(wz[:], 0.0)
            warm = psum_w.tile([128, 128], f32)
            for i in range(WARMUP_MM):
                nc.tensor.matmul(warm[:], wz[:], wz[:],
                                 start=(i == 0), stop=(i == WARMUP_MM - 1))

            # head one-hots + iota on the scalar HWDGE ring (fast, idle at
            # start); bulk one-hots and other consts on gpsimd (SWDGE) as
            # single large transfers (small slices would flood the SDMA
            # engines with sub-KB descriptors and starve the stream)
            n_head = sum(1 for s in src[:max(CH0, N_HEAD_OH)] if s == 'h')
            oh_head_t = oh_tail_t = None
            if n_head:
                oh_head_t = consts.tile([128, n_head * 128], f8e3)
                # first 8 tiles in their own transfer so the opening
                # matmuls have weights as early as possible
                n0 = min(8, n_head)
                nc.scalar.dma_start(oh_head_t[:, 0:n0 * 128],
                                    oh_d[:, 0:n0 * 128])
                if n_head > n0:
                    nc.scalar.dma_start(oh_head_t[:, n0 * 128:n_head * 128],
                                        oh_d[:, n0 * 128:n_head * 128])
            iota_t = consts.tile([128, 128], f16)
            nc.scalar.dma_start(iota_t[:], iota[:])
            cst_t = consts.tile([128, CW], f32)
            nc.gpsimd.dma_start(cst_t[:], cst32[:])
            if n_h > n_head:
                oh_tail_t = consts.tile([128, (n_h - n_head) * 128], f8e3)
                nc.gpsimd.dma_start(oh_tail_t[:],
                                    oh_d[:, n_head * 128:n_h * 128])
            wt_t = consts.tile([128, 2 * C], f16)
            nc.gpsimd.dma_start(wt_t[:], wt[:])
            relT = cst_t[:, 0:T]
            b_t = cst_t[:, 2 * T:2 * T + C]
            recip_t = cst_t[:, 2 * T + C:2 * T + C + BLOCKS_PER_CORE]
            ident_t = consts.tile([128, 128], f32)
            masks.make_identity(nc, ident_t[:])

            pend_t = []    # blocks awaiting PE transpose
            pend_c = []    # blocks awaiting classifier

            def stage_t(item):
                j, sums_t = item
                sT = []
                for q in range(2):
                    p_t = psum_t.tile([128, 128], f32, tag="psum_t")
                    nc.tensor.transpose(
                        p_t[:], sums_t[:, q * 128:(q + 1) * 128], ident_t[:])
                    s_t = sT_pool.tile([128, 128], f16, tag="sT")
                    nc.scalar.copy(s_t[:], p_t[:])
                    sT.append(s_t)
                pend_c.append((j, sT[0], sT[1]))

            def stage_c(item):
                j, sT0, sT1 = item
                po = psum_o.tile([128, C], f32, tag="po")
                nc.tensor.matmul(po[:], sT0[:], wt_t[:, 0:C],
                                 start=True, stop=False)
                nc.tensor.matmul(po[:], sT1[:], wt_t[:, C:2 * C],
                                 start=False, stop=True)
                ob = ob_pool.tile([128, C], mybir.dt.bfloat16, tag="ob")
                nc.vector.scalar_tensor_tensor(
                    ob[:], po[:], recip_t[:, j:j + 1], b_t,
                    mybir.AluOpType.mult, mybir.AluOpType.add)
                nc.scalar.dma_start(out[j], ob[:])

            psum_cur = None
            for c, (t0, t1) in enumerate(chunks):
                L = t1 - t0
                hid_t = hid_pool.tile([128, L * H], f8e3, tag="hid")
                dma_eng = nc.sync if (c % 2 == 0) else nc.scalar
                if c == 0:
                    # slice so the opening tiles start as soon as they land
                    nsl = 8
                    step = (L * H) // nsl
                    for sl in range(nsl):
                        dma_eng.dma_start(
                            hid_t[:, sl * step:(sl + 1) * step],
                            hid[:, t0 * H + sl * step:t0 * H + (sl + 1) * step])
                else:
                    dma_eng.dma_start(hid_t[:], hid[:, t0 * H:t1 * H])

                for t in range(t0, t1):
                    j = bisect.bisect_right(offs, t) - 1
                    i = t - offs[j]
                    tb = pos_tblks[j]

                    if src[t] == 'h':
                        slot = int(host_slot[t])
                        if slot < n_head:
                            lhsT = oh_head_t[:, slot * 128:(slot + 1) * 128]
                        else:
                            sl = slot - n_head
                            lhsT = oh_tail_t[:, sl * 128:(sl + 1) * 128]
                    else:
                        a_t = a_pool.tile([128, 128], f16, tag="onehot")
                        nc.vector.tensor_scalar(
                            a_t[:], iota_t[:], relT[:, t:t + 1], None,
                            mybir.AluOpType.is_equal)
                        lhsT = a_t[:]

                    if i == 0:
                        psum_cur = psum_s.tile([128, H], f32, tag="psum_s")
                    nc.tensor.matmul(
                        psum_cur[:], lhsT, hid_t[:, (t - t0) * H:(t - t0 + 1) * H],
                        start=(i == 0), stop=(i == tb - 1))

                    if i == tb - 1:
                        sums_t = sums_pool.tile([128, H], f32, tag="sums")
                        nc.scalar.copy(sums_t[:], psum_cur[:])
                        pend_t.append((j, sums_t))
                        if len(pend_t) > 1:
                            stage_t(pend_t.pop(0))
                        if len(pend_c) > 1:
                            stage_c(pend_c.pop(0))
            while pend_t:
                stage_t(pend_t.pop(0))
            while pend_c:
                stage_c(pend_c.pop(0))
    nc.compile()
    return nc, src


def _quantize_ef(hidden, bag_id):
    """fp8 E3M4 with per-(bag, h) error feedback down the rows."""
    edges = np.searchsorted(bag_id, np.arange(NUM_BAGS + 1))
    starts = edges[:-1]
    lens = np.diff(edges)
    hq = np.zeros((N, H), F8)
    carry = np.zeros((NUM_BAGS, H), np.float32)
    for k in range(int(lens.max())):
        m = lens > k
        idx = starts[m] + k
        v = hidden[idx] + carry[m]
        q = v.astype(F8)
        hq[idx] = q
        carry[m] = v - q.astype(np.float32)
    return hq, edges


def kernel(hidden, W, b, bag_id):
    global LAST_RESULTS
    hidden = np.asarray(hidden, dtype=np.float32)
    W = np.asarray(W, dtype=np.float32)
    b = np.asarray(b, dtype=np.float32)
    bag_id = np.asarray(bag_id).astype(np.int64)

    counts = np.bincount(bag_id, minlength=NUM_BAGS)
    recip_all = (1.0 / np.maximum(counts, 1)).astype(np.float32)

    hq, bag_edges = _quantize_ef(hidden, bag_id)

    nblocks = NUM_BAGS // BLOCK_BAGS                     # 64
    edges = bag_edges[::BLOCK_BAGS]                      # block row edges
    blk_len = np.diff(edges)
    tiles_per_blk = np.maximum(1, -(-blk_len // 128))
    pos_tblks = tuple(
        int(x) for x in
        tiles_per_blk.reshape(NCORES, BLOCKS_PER_CORE).max(axis=0))
    T = sum(pos_tblks)
    offs = np.concatenate([[0], np.cumsum(pos_tblks)])

    # padded per-(core, position) rows + relative bag ids
    xp8 = np.zeros((NCORES, T * 128, H), F8)
    rel = np.full((NCORES, T * 128), -1.0, dtype=np.float32)
    for bidx in range(nblocks):
        k, j = divmod(bidx, BLOCKS_PER_CORE)
        s, e = int(edges[bidx]), int(edges[bidx + 1])
        ln = e - s
        r0 = int(offs[j]) * 128
        if ln:
            xp8[k, r0:r0 + ln] = hq[s:e]
            rel[k, r0:r0 + ln] = (bag_id[s:e] - bidx * BLOCK_BAGS).astype(
                np.float32)

    src = _tile_assignment(T)
    n_h = sum(1 for s in src if s == 'h')

    wt_np = np.ascontiguousarray(W.T).astype(np.float16)      # [256, 128]
    wt_packed = np.concatenate([wt_np[0:128], wt_np[128:256]],
                               axis=1)                        # [128, 2C] f16
    b_np = np.tile(b, (128, 1)).astype(np.float32)
    iota_np = np.tile(np.arange(128, dtype=np.float16), (128, 1))

    in_maps = []
    for k in range(NCORES):
        relc = rel[k].reshape(T, 128).T                       # [128, T]
        recc = recip_all[k * 1024:(k + 1) * 1024].reshape(
            BLOCKS_PER_CORE, 128).T                           # [128, 8]
        cst_np = np.concatenate(
            [relc, -relc, b_np, recc], axis=1).astype(np.float32)
        hidc = np.ascontiguousarray(
            xp8[k].reshape(T, 128, H).transpose(1, 0, 2).reshape(128, T * H))
        m = {"hid": hidc, "cst32": np.ascontiguousarray(cst_np),
             "wt": np.ascontiguousarray(wt_packed), "iota": iota_np}
        if n_h:
            oh_np = np.zeros((128, n_h, 128), F8)
            slot = 0
            rk = rel[k].reshape(T, 128)
            for t in range(T):
                if src[t] == 'h':
                    rr = rk[t].astype(np.int32)
                    valid = rr >= 0
                    oh_np[np.arange(128)[valid], slot, rr[valid]] = 1.0
                    slot += 1
            m["oh"] = np.ascontiguousarray(oh_np.reshape(128, n_h * 128))
        in_maps.append(m)

    key = (pos_tblks, CH, CH0, N_HOST_OH, N_HEAD_OH, WARMUP_MM)
    if key not in _prog_cache:
        _prog_cache[key] = _build_program(pos_tblks)
    nc, _ = _prog_cache[key]

    trace = False
    if os.environ.get("BASS_TRACE"):
        trace = _install_ntff_shim()

    res = run_bass_kernel_spmd(nc, in_maps, core_ids=list(range(NCORES)),
                               trace=trace)
    LAST_RESULTS = res

    out = np.concatenate(
        [np.asarray(res.results[k]["out"]).astype(np.float32).reshape(1024, C)
         for k in range(NCORES)], axis=0)
    return out


# revision 29
# speedup vs baseline: 1.0327x; 1.0327x over previous
"""BagRE segment-mean + classifier kernel for 8 Trainium2 NeuronCores (fp8).

Problem:  hidden [262144, 256] f32, sorted bag_id [262144] i64 with 8192 bags,
          W [128, 256], b [128]  ->  logits [8192, 128] f32
          logits = (segment_mean(hidden, bag_id) @ W.T) + b

Strategy:
  bag_id is sorted -> rows per bag are contiguous.  Core k owns bags
  [1024k, 1024(k+1)), split into 8 blocks of 128 bags; block rows are padded
  to whole 128-row tiles (per-position max over cores so all 8 cores run one
  SPMD program).

  The hidden stream is quantized host-side to fp8 E3M4 with error feedback
  down each (bag, h) column, so the bag-sum error telescopes to one quantum
  (~4e-3 end-to-end rel err) while DMA bytes halve vs fp16.  Stream chunks
  alternate between the two HWDGE rings (sync / scalar) for double issue
  throughput.

  Per 128-row tile the PE accumulates one_hot(rel).T @ X into PSUM
  [128 bags, 256] f32.  One-hots (f16) come from DVE is_equal for most
  tiles, with a host-precomputed fp8 subset (single large upfront DMA) to
  keep the DVE off the critical path; the head of the stream is all-host so
  the PE starts before the DVE consts land.  ~16 warmup matmuls on a zero
  tile hold the PE HAM clock at 2.4 GHz through the DMA ramp.

  Finalize is a 3-stage pipeline, each stage one block behind the stream so
  the PE never waits on cross-engine latency: block j's PSUM sums are
  copied to SBUF (ACT) at block j's end; PE-transposed to [h, bags] (f16)
  at block j+1's end; classifier GEMM + fused recip/bias (DVE) + bf16
  store at block j+2's end.
"""

import os
import sys
import bisect
import contextlib
import numpy as np

try:
    import concourse.bass as bass  # noqa: F401
except Exception:  # pragma: no cover
    sys.path.insert(0, "/opt/trn_rl_repo")

import ml_dtypes
import concourse.bass as bass
import concourse.tile as tile
from concourse import mybir, bacc, masks
from concourse.bass_utils import run_bass_kernel_spmd

F8 = ml_dtypes.float8_e3m4

N = 262144
H = 256
C = 128
NUM_BAGS = 8192
NCORES = 8
BLOCK_BAGS = 128
BLOCKS_PER_CORE = NUM_BAGS // BLOCK_BAGS // NCORES   # 8

CH = int(os.environ.get("BK_CH", "16"))              # tiles per stream chunk
CH0 = int(os.environ.get("BK_CH0", "8"))             # first two (short) chunks
N_HOST_OH = int(os.environ.get("BK_NHOST", "112"))   # host-supplied one-hots
N_HEAD_OH = int(os.environ.get("BK_NHEAD", "24"))    # host one-hots up front
WARMUP_MM = int(os.environ.get("BK_WARMUP", "28"))

LAST_RESULTS = None
_prog_cache = {}


def _install_ntff_shim():
    """Register the axon NTFF profiling hook so trace=True works."""
    try:
        from antenv.axon_hooks import get_axon_ntff_profile_hook  # noqa: F401
        return True
    except Exception:
        pass
    try:
        import types
        import antenv
        from trn_agent_boot.trn_boot import _ntff_profile_via_ctypes

        hook = _ntff_profile_via_ctypes("/opt/axon/libaxon_pjrt.so")
        if hook is None:
            return False
        mod = types.ModuleType("antenv.axon_hooks")
        mod._hook = hook
        mod.get_axon_ntff_profile_hook = lambda: mod._hook
        mod.set_axon_ntff_profile_hook = lambda h: setattr(mod, "_hook", h)
        sys.modules["antenv.axon_hooks"] = mod
        antenv.axon_hooks = mod
        import concourse.bass_utils as bu

        orig_upload = bu.upload_artifacts

        def _safe_upload(tmpdir):
            try:
                return orig_upload(tmpdir)
            except Exception:
                return tmpdir

        bu.upload_artifacts = _safe_upload
        return True
    except Exception:
        return False


def _tile_assignment(T):
    """Per-tile one-hot source: 'h' host, 'v' DVE.

    The first N_HEAD_OH tiles are host tiles so the PE can start before the
    DVE's consts have landed; the rest spread evenly over the tail.
    """
    src = ['v'] * T
    nh = min(N_HOST_OH, T)
    head = min(N_HEAD_OH, nh)
    for t in range(head):
        src[t] = 'h'
    rest = nh - head
    Tr = T - head
    if rest > 0:
        for q in range(Tr):
            if (q * rest) // Tr != ((q + 1) * rest) // Tr:
                src[head + q] = 'h'
    return tuple(src)


def _build_program(pos_tblks):
    T = sum(pos_tblks)
    offs = [0]
    for tb in pos_tblks:
        offs.append(offs[-1] + tb)
    src = _tile_assignment(T)
    host_slot = np.cumsum([1 if s == 'h' else 0 for s in src]) - 1
    chunks = [(0, min(CH0, T))]
    if chunks[-1][1] < T:
        chunks.append((chunks[-1][1], min(chunks[-1][1] + CH0, T)))
    while chunks[-1][1] < T:
        chunks.append((chunks[-1][1], min(chunks[-1][1] + CH, T)))

    f32 = mybir.dt.float32
    f16 = mybir.dt.float16
    bf16 = mybir.dt.bfloat16
    f8e3 = mybir.dt.float8e3

    nc = bacc.Bacc(trn_type="TRN2", target_bir_lowering=False, debug=False)
    hid = nc.dram_tensor("hid", [128, T * H], f8e3, kind="ExternalInput").ap()
    n_h = sum(1 for s in src if s == 'h')
    oh_d = None
    if n_h:
        oh_d = nc.dram_tensor("oh", [128, n_h * 128], f8e3,
                              kind="ExternalInput").ap()
    # cst32: [relT (T) | nrelT (T) | b (C) | recip (8)]
    CW = 2 * T + C + BLOCKS_PER_CORE
    cst32 = nc.dram_tensor("cst32", [128, CW], f32, kind="ExternalInput").ap()
    wt = nc.dram_tensor("wt", [128, 2 * C], f16, kind="ExternalInput").ap()
    iota = nc.dram_tensor("iota", [128, 128], f16, kind="ExternalInput").ap()
    out = nc.dram_tensor("out", [BLOCKS_PER_CORE, 128, C], bf16,
                         kind="ExternalOutput").ap()

    with tile.TileContext(nc) as tc:
        with contextlib.ExitStack() as ctx:
            consts = ctx.enter_context(tc.tile_pool(name="consts", bufs=1))
            hid_pool = ctx.enter_context(tc.tile_pool(name="hid", bufs=10))
            a_pool = ctx.enter_context(tc.tile_pool(name="onehot", bufs=12))
            sums_pool = ctx.enter_context(tc.tile_pool(name="sums", bufs=4))
            sT_pool = ctx.enter_context(tc.tile_pool(name="sT", bufs=6))
            ob_pool = ctx.enter_context(tc.tile_pool(name="ob", bufs=3))
            psum_s = ctx.enter_context(
                tc.tile_pool(name="psum_s", bufs=3, space="PSUM"))
            psum_t = ctx.enter_context(
                tc.tile_pool(name="psum_t", bufs=2, space="PSUM"))
            psum_o = ctx.enter_context(
                tc.tile_pool(name="psum_o", bufs=2, space="PSUM"))
            psum_w = ctx.enter_context(
                tc.tile_pool(name="psum_w", bufs=1, space="PSUM"))

            # --- PE warmup: keep HAM at 2.4 GHz while DMA ramps ---------
            wz = consts.tile([128, 128], f16)
            nc.vector.memset(wz[:], 0.0)
            warm = psum_w.tile([128, 128], f32)
            for i in range(WARMUP_MM):
                nc.tensor.matmul(warm[:], wz[:], wz[:],
                                 start=(i == 0), stop=(i == WARMUP_MM - 1))

            # head one-hots + iota on the scalar HWDGE ring (fast, idle at
            # start); bulk one-hots and other consts on gpsimd (SWDGE) as
            # single large transfers (small slices would flood the SDMA
            # engines with sub-KB descriptors and starve the stream)
            n_head = sum(1 for s in src[:max(CH0, N_HEAD_OH)] if s == 'h')
            oh_head_t = oh_tail_t = None
            if n_head:
                oh_head_t = consts.tile([128, n_head * 128], f8e3)
                # first 8 tiles in their own transfer so the opening
                # matmuls have weights as early as possible
                n0 = min(8, n_head)
                nc.scalar.dma_start(oh_head_t[:, 0:n0 * 128],
                                    oh_d[:, 0:n0 * 128])
                if n_head > n0:
                    nc.scalar.dma_start(oh_head_t[:, n0 * 128:n_head * 128],
                                        oh_d[:, n0 * 128:n_head * 128])
            iota_t = consts.tile([128, 128], f16)
            nc.scalar.dma_start(iota_t[:], iota[:])
            cst_t = consts.tile([128, CW], f32)
            nc.gpsimd.dma_start(cst_t[:], cst32[:])
            if n_h > n_head:
                oh_tail_t = consts.tile([128, (n_h - n_head) * 128], f8e3)
                nc.gpsimd.dma_start(oh_tail_t[:],
                                    oh_d[:, n_head * 128:n_h * 128])
            wt_t = consts.tile([128, 2 * C], f16)
            nc.gpsimd.dma_start(wt_t[:], wt[:])
            relT = cst_t[:, 0:T]
            b_t = cst_t[:, 2 * T:2 * T + C]
            recip_t = cst_t[:, 2 * T + C:2 * T + C + BLOCKS_PER_CORE]
            ident_t = consts.tile([128, 128], f32)
            masks.make_identity(nc, ident_t[:])

            pend_t = []    # blocks awaiting PE transpose
            pend_c = []    # blocks awaiting classifier

            def stage_t(item):
                j, sums_t = item
                sT = []
                for q in range(2):
                    p_t = psum_t.tile([128, 128], f32, tag="psum_t")
                    nc.tensor.transpose(
                        p_t[:], sums_t[:, q * 128:(q + 1) * 128], ident_t[:])
                    s_t = sT_pool.tile([128, 128], f16, tag="sT")
                    nc.scalar.copy(s_t[:], p_t[:])
                    sT.append(s_t)
                pend_c.append((j, sT[0], sT[1]))

            def stage_c(item):
                j, sT0, sT1 = item
                po = psum_o.tile([128, C], f32, tag="po")
                nc.tensor.matmul(po[:], sT0[:], wt_t[:, 0:C],
                                 start=True, stop=False)
                nc.tensor.matmul(po[:], sT1[:], wt_t[:, C:2 * C],
                                 start=False, stop=True)
                ob = ob_pool.tile([128, C], mybir.dt.bfloat16, tag="ob")
                nc.vector.scalar_tensor_tensor(
                    ob[:], po[:], recip_t[:, j:j + 1], b_t,
                    mybir.AluOpType.mult, mybir.AluOpType.add)
                nc.scalar.dma_start(out[j], ob[:])

            psum_cur = None
            for c, (t0, t1) in enumerate(chunks):
                L = t1 - t0
                hid_t = hid_pool.tile([128, L * H], f8e3, tag="hid")
                dma_eng = nc.sync if (c % 2 == 0) else nc.scalar
                if c == 0:
                    # slice so the opening tiles start as soon as they land
                    nsl = 8
                    step = (L * H) // nsl
                    for sl in range(nsl):
                        dma_eng.dma_start(
                            hid_t[:, sl * step:(sl + 1) * step],
                            hid[:, t0 * H + sl * step:t0 * H + (sl + 1) * step])
                else:
                    dma_eng.dma_start(hid_t[:], hid[:, t0 * H:t1 * H])

                for t in range(t0, t1):
                    j = bisect.bisect_right(offs, t) - 1
                    i = t - offs[j]
                    tb = pos_tblks[j]

                    if src[t] == 'h':
                        slot = int(host_slot[t])
                        if slot < n_head:
                            lhsT = oh_head_t[:, slot * 128:(slot + 1) * 128]
                        else:
                            sl = slot - n_head
                            lhsT = oh_tail_t[:, sl * 128:(sl + 1) * 128]
                    else:
                        a_t = a_pool.tile([128, 128], f16, tag="onehot")
                        nc.vector.tensor_scalar(
                            a_t[:], iota_t[:], relT[:, t:t + 1], None,
                            mybir.AluOpType.is_equal)
                        lhsT = a_t[:]

                    if i == 0:
                        psum_cur = psum_s.tile([128, H], f32, tag="psum_s")
                    nc.tensor.matmul(
                        psum_cur[:], lhsT, hid_t[:, (t - t0) * H:(t - t0 + 1) * H],
                        start=(i == 0), stop=(i == tb - 1))

                    if i == tb - 1:
                        sums_t = sums_pool.tile([128, H], f32, tag="sums")
                        nc.scalar.copy(sums_t[:], psum_cur[:])
                        pend_t.append((j, sums_t))
                        if len(pend_t) > 1:
                            stage_t(pend_t.pop(0))
                        if len(pend_c) > 1:
                            stage_c(pend_c.pop(0))
            while pend_t:
                stage_t(pend_t.pop(0))
            while pend_c:
                stage_c(pend_c.pop(0))
    nc.compile()
    return nc, src


def _quantize_ef(hidden, bag_id):
    """fp8 E3M4 with per-(bag, h) error feedback down the rows."""
    edges = np.searchsorted(bag_id, np.arange(NUM_BAGS + 1))
    starts = edges[:-1]
    lens = np.diff(edges)
    hq = np.zeros((N, H), F8)
    carry = np.zeros((NUM_BAGS, H), np.float32)
    for k in range(int(lens.max())):
        m = lens > k
        idx = starts[m] + k
        v = hidden[idx] + carry[m]
        q = v.astype(F8)
        hq[idx] = q
        carry[m] = v - q.astype(np.float32)
    return hq, edges


def kernel(hidden, W, b, bag_id):
    global LAST_RESULTS
    hidden = np.asarray(hidden, dtype=np.float32)
    W = np.asarray(W, dtype=np.float32)
    b = np.asarray(b, dtype=np.float32)
    bag_id = np.asarray(bag_id).astype(np.int64)

    counts = np.bincount(bag_id, minlength=NUM_BAGS)
    recip_all = (1.0 / np.maximum(counts, 1)).astype(np.float32)

    hq, bag_edges = _quantize_ef(hidden, bag_id)

    nblocks = NUM_BAGS // BLOCK_BAGS                     # 64
    edges = bag_edges[::BLOCK_BAGS]                      # block row edges
    blk_len = np.diff(edges)
    tiles_per_blk = np.maximum(1, -(-blk_len // 128))
    pos_tblks = tuple(
        int(x) for x in
        tiles_per_blk.reshape(NCORES, BLOCKS_PER_CORE).max(axis=0))
    T = sum(pos_tblks)
    offs = np.concatenate([[0], np.cumsum(pos_tblks)])

    # padded per-(core, position) rows + relative bag ids
    xp8 = np.zeros((NCORES, T * 128, H), F8)
    rel = np.full((NCORES, T * 128), -1.0, dtype=np.float32)
    for bidx in range(nblocks):
        k, j = divmod(bidx, BLOCKS_PER_CORE)
        s, e = int(edges[bidx]), int(edges[bidx + 1])
        ln = e - s
        r0 = int(offs[j]) * 128
        if ln:
            xp8[k, r0:r0 + ln] = hq[s:e]
            rel[k, r0:r0 + ln] = (bag_id[s:e] - bidx * BLOCK_BAGS).astype(
                np.float32)

    src = _tile_assignment(T)
    n_h = sum(1 for s in src if s == 'h')

    wt_np = np.ascontiguousarray(W.T).astype(np.float16)      # [256, 128]
    wt_packed = np.concatenate([wt_np[0:128], wt_np[128:256]],
                               axis=1)                        # [128, 2C] f16
    b_np = np.tile(b, (128, 1)).astype(np.float32)
    iota_np = np.tile(np.arange(128, dtype=np.float16), (128, 1))

    in_maps = []
    for k in range(NCORES):
        relc = rel[k].reshape(T, 128).T                       # [128, T]
        recc = recip_all[k * 1024:(k + 1) * 1024].reshape(
            BLOCKS_PER_CORE, 128).T                           # [128, 8]
        cst_np = np.concatenate(
            [relc, -relc, b_np, recc], axis=1).astype(np.float32)
        hidc = np.ascontiguousarray(
            xp8[k].reshape(T, 128, H).transpose(1, 0, 2).reshape(128, T * H))
        m = {"hid": hidc, "cst32": np.ascontiguousarray(cst_np),
             "wt": np.ascontiguousarray(wt_packed), "iota": iota_np}
        if n_h:
            oh_np = np.zeros((128, n_h, 128), F8)
            slot = 0
            rk = rel[k].reshape(T, 128)
            for t in range(T):
                if src[t] == 'h':
                    rr = rk[t].astype(np.int32)
                    valid = rr >= 0
                    oh_np[np.arange(128)[valid], slot, rr[valid]] = 1.0
                    slot += 1
            m["oh"] = np.ascontiguousarray(oh_np.reshape(128, n_h * 128))
        in_maps.append(m)

    key = (pos_tblks, CH, CH0, N_HOST_OH, N_HEAD_OH, WARMUP_MM)
    if key not in _prog_cache:
        _prog_cache[key] = _build_program(pos_tblks)
    nc, _ = _prog_cache[key]

    trace = False
    if os.environ.get("BASS_TRACE"):
        trace = _install_ntff_shim()

    res = run_bass_kernel_spmd(nc, in_maps, core_ids=list(range(NCORES)),
                               trace=trace)
    LAST_RESULTS = res

    out = np.concatenate(
        [np.asarray(res.results[k]["out"]).astype(np.float32).reshape(1024, C)
         for k in range(NCORES)], axis=0)
    return out
